# revision 7
# baseline (speedup 1.0000x reference)
"""Trainium2 Bass kernel for Llama attention (B=2, S=2048, DIM=2048, 16 heads).

Sharding: tensor-parallel over heads x data-parallel over batch.
Core c (0..7): batch = c//4, head-quad hb = c%4 (heads hb*4 .. hb*4+3).
Each core computes q/k/v projections + RoPE + attention + its partial o_proj
(columns of 4 heads); the host sums 4 partials per batch.

All matmuls run in float32r (full PE rate, ~1e-4 relative precision).
Layouts (per core):
  xT      [DIM, S]    hidden_states[b].T           (streamed 3x, once per proj phase)
  wqT/wkT [DIM, 512]  wq[rows].T for the 4 heads   (rows = hb*512..hb*512+512)
  wvT     [DIM, 512]
  woT     [512, DIM]  wo[:, cols].T
  cosT/sinT [128, S]  rotary cache transposed; sinT pre-signed for rotate_half
  out     [S, DIM]    partial output (natural layout)

On-chip: qT/kT [hd=128, S] per head (scores contract over hd on partitions);
v natural [S-tiles, 4*128]; scoresT [sk, sq] via kT-stationary matmul; softmax
along partitions via exp (ACT, scale folded) + all-ones-stationary matmul
(yields denominator broadcast to all 128 partitions); attnT = pv_psum * recip;
o_proj with attnT stationary -> natural-layout partial.
"""

import sys

sys.path.insert(0, "/opt/trn_rl_repo")

import numpy as np
import ml_dtypes

import concourse.mybir as mybir
import concourse.tile as tile
from concourse import bacc

DIM = 2048
S = 2048
B = 2
N_HEADS = 16
HD = 128
N_CORES = 8
HPC = 4          # heads per core
HB = HPC * HD    # head-block columns per core (512)
ROPE_BASE = 10000.0
SCALE = 1.0 / np.sqrt(HD)

MM_DT = mybir.dt.float32r   # matmul operand dtype
MM_NP = np.float32
F32 = mybir.dt.float32

KC = DIM // 128   # 16 contraction chunks
NB = 256          # S-block for q/k projection moving operand
SQB = 1024        # sq block in attention (exp granularity, 2 psum banks)


def _build_module():
    nc = bacc.Bacc(None, target_bir_lowering=False)

    xT_d = nc.dram_tensor("xT", [DIM, S], MM_DT, kind="ExternalInput")
    wqT_d = nc.dram_tensor("wqT", [DIM, HB], MM_DT, kind="ExternalInput")
    wkT_d = nc.dram_tensor("wkT", [DIM, HB], MM_DT, kind="ExternalInput")
    wvT_d = nc.dram_tensor("wvT", [DIM, HB], MM_DT, kind="ExternalInput")
    woT_d = nc.dram_tensor("woT", [HB, DIM], MM_DT, kind="ExternalInput")
    cos_d = nc.dram_tensor("cosT", [HD, S], F32, kind="ExternalInput")
    sin_d = nc.dram_tensor("sinT", [HD, S], F32, kind="ExternalInput")
    ones_d = nc.dram_tensor("ones", [128, 128], MM_DT, kind="ExternalInput")
    out_d = nc.dram_tensor("out", [S, DIM], F32, kind="ExternalOutput")

    xT_r = xT_d.rearrange("(k p) s -> p k s", p=128)       # [128, KC, S]
    wqT_r = wqT_d.rearrange("(k p) m -> p k m", p=128)     # [128, KC, 512]
    wkT_r = wkT_d.rearrange("(k p) m -> p k m", p=128)
    wvT_r = wvT_d.rearrange("(k p) m -> p k m", p=128)
    woT_r = woT_d.rearrange("(k p) m -> p k m", p=128)     # [128, 4, DIM]

    with tile.TileContext(nc) as tc:
        with (
            tc.tile_pool(name="qk", bufs=1) as qk_pool,
            tc.tile_pool(name="vna", bufs=1) as v_pool,
            tc.tile_pool(name="misc", bufs=1) as misc_pool,
        ):
            # persistent tiles
            qk_sb = qk_pool.tile([128, 2, HPC, S], MM_DT)    # q/k heads, [hd, S]
            v_sb = v_pool.tile([128, S // 128, HB], MM_DT)   # v natural
            ones_sb = misc_pool.tile([128, 128], MM_DT)
            nc.sync.dma_start(out=ones_sb, in_=ones_d[:, :])

            # ---------------- P1q / P1k: q,k projections + RoPE --------------
            with tc.tile_pool(name="cs", bufs=1) as cs_pool:
                cos_sb = cs_pool.tile([128, S], F32)
                sin_sb = cs_pool.tile([128, S], F32)
                nc.sync.dma_start(out=cos_sb, in_=cos_d[:, :])
                nc.sync.dma_start(out=sin_sb, in_=sin_d[:, :])

                for t, w_r in ((0, wqT_r), (1, wkT_r)):
                    with (
                        tc.tile_pool(name=f"w{t}", bufs=1) as w_pool,
                        tc.tile_pool(name=f"x{t}", bufs=2) as x_pool,
                        tc.tile_pool(name=f"pp{t}", bufs=2, space="PSUM") as pp,
                        tc.tile_pool(name=f"rt{t}", bufs=3) as rt_pool,
                    ):
                        w_sb = w_pool.tile([128, KC, HB], MM_DT)
                        nc.sync.dma_start(out=w_sb, in_=w_r[:, :, :])
                        for nb in range(S // NB):
                            x_sb = x_pool.tile([128, KC, NB], MM_DT, tag="x")
                            nc.sync.dma_start(
                                out=x_sb, in_=xT_r[:, :, nb * NB:(nb + 1) * NB]
                            )
                            for h in range(HPC):
                                ps = pp.tile([128, NB], F32, tag="ps")
                                for kc in range(KC):
                                    nc.tensor.matmul(
                                        ps[:, :],
                                        w_sb[:, kc, h * HD:(h + 1) * HD],
                                        x_sb[:, kc, :],
                                        start=(kc == 0),
                                        stop=(kc == KC - 1),
                                    )
                                # RoPE: qrot = q*cos + rot_half(q)*sin_signed
                                nslice = slice(nb * NB, (nb + 1) * NB)
                                t1 = rt_pool.tile([128, NB], F32, tag="t1")
                                t2 = rt_pool.tile([128, NB], F32, tag="t2")
                                nc.vector.tensor_mul(
                                    t1[:, :], ps[:, :], cos_sb[:, nslice]
                                )
                                nc.vector.tensor_mul(
                                    t2[0:64, :], ps[64:128, :], sin_sb[0:64, nslice]
                                )
                                nc.vector.tensor_mul(
                                    t2[64:128, :], ps[0:64, :], sin_sb[64:128, nslice]
                                )
                                nc.vector.tensor_add(
                                    qk_sb[:, t, h, nslice], t1[:, :], t2[:, :]
                                )

            # ---------------- P1v: v projection (natural layout) -------------
            with (
                tc.tile_pool(name="wv", bufs=1) as wv_pool,
                tc.tile_pool(name="xv", bufs=3) as xv_pool,
                tc.tile_pool(name="ppv", bufs=2, space="PSUM") as ppv,
            ):
                wv_sb = wv_pool.tile([128, KC, HB], MM_DT)
                nc.sync.dma_start(out=wv_sb, in_=wvT_r[:, :, :])
                for m in range(S // 128):
                    xm = xv_pool.tile([128, KC, 128], MM_DT, tag="xm")
                    nc.sync.dma_start(
                        out=xm, in_=xT_r[:, :, m * 128:(m + 1) * 128]
                    )
                    ps = ppv.tile([128, HB], F32, tag="psv")
                    for kc in range(KC):
                        nc.tensor.matmul(
                            ps[:, :],
                            xm[:, kc, :],
                            wv_sb[:, kc, :],
                            start=(kc == 0),
                            stop=(kc == KC - 1),
                        )
                    nc.scalar.copy(v_sb[:, m, :], ps[:, :])

            # ---------------- P2: attention -----------------------------------
            with (
                tc.tile_pool(name="attn", bufs=1) as attn_pool,
                tc.tile_pool(name="wo", bufs=1) as wo_pool,
            ):
              attn_sb = attn_pool.tile([128, HPC, S], MM_DT)   # attnT per head
              # preload wo during attention
              wo_sb = wo_pool.tile([128, HPC, DIM], MM_DT)
              nc.sync.dma_start(out=wo_sb, in_=woT_r[:, :, :])
              with (
                tc.tile_pool(name="pr", bufs=3) as pr_pool,
                tc.tile_pool(name="rc", bufs=2) as rc_pool,
                tc.tile_pool(name="sp", bufs=2, space="PSUM") as sp,
                tc.tile_pool(name="pvp", bufs=1, space="PSUM") as pvp,
                tc.tile_pool(name="dnp", bufs=1, space="PSUM") as dnp,
              ):
                for h in range(HPC):
                    for nq in range(S // SQB):
                        sqs = slice(nq * SQB, (nq + 1) * SQB)
                        pv_ps = pvp.tile([128, SQB], F32, tag="pv")
                        dn_ps = dnp.tile([128, SQB], F32, tag="dn")
                        for sk in range(S // 128):
                            s_ps = sp.tile([128, SQB], F32, tag="s")
                            for half in range(SQB // 512):
                                hs = slice(half * 512, (half + 1) * 512)
                                qs = slice(nq * SQB + half * 512,
                                           nq * SQB + (half + 1) * 512)
                                nc.tensor.matmul(
                                    s_ps[:, hs],
                                    qk_sb[:, 1, h, sk * 128:(sk + 1) * 128],
                                    qk_sb[:, 0, h, qs],
                                    start=True,
                                    stop=True,
                                )
                            probs = pr_pool.tile([128, SQB], MM_DT, tag="pr")
                            nc.scalar.activation(
                                probs[:, :], s_ps[:, :],
                                mybir.ActivationFunctionType.Exp,
                                scale=float(SCALE),
                            )
                            for half in range(SQB // 512):
                                hs = slice(half * 512, (half + 1) * 512)
                                nc.tensor.matmul(
                                    pv_ps[:, hs],
                                    v_sb[:, sk, h * HD:(h + 1) * HD],
                                    probs[:, hs],
                                    start=(sk == 0),
                                    stop=(sk == S // 128 - 1),
                                )
                                nc.tensor.matmul(
                                    dn_ps[:, hs],
                                    ones_sb[:, :],
                                    probs[:, hs],
                                    start=(sk == 0),
                                    stop=(sk == S // 128 - 1),
                                )
                        recip = rc_pool.tile([128, SQB], F32, tag="rc")
                        nc.vector.reciprocal(recip[:, :], dn_ps[:, :])
                        nc.vector.tensor_mul(
                            attn_sb[:, h, sqs], pv_ps[:, :], recip[:, :]
                        )

              # ---------------- P3: o_proj ------------------------------------
              with (
                tc.tile_pool(name="op", bufs=4, space="PSUM") as op,
                tc.tile_pool(name="ost", bufs=4) as ost_pool,
              ):
                for m in range(S // 128):
                    for n in range(DIM // 512):
                        ps = op.tile([128, 512], F32, tag="o")
                        for h in range(HPC):
                            nc.tensor.matmul(
                                ps[:, :],
                                attn_sb[:, h, m * 128:(m + 1) * 128],
                                wo_sb[:, h, n * 512:(n + 1) * 512],
                                start=(h == 0),
                                stop=(h == HPC - 1),
                            )
                        o = ost_pool.tile([128, 512], F32, tag="ot")
                        nc.scalar.copy(o[:, :], ps[:, :])
                        nc.sync.dma_start(
                            out=out_d[m * 128:(m + 1) * 128, n * 512:(n + 1) * 512],
                            in_=o[:, :],
                        )

    nc.finalize()
    return nc


# ---------------------------------------------------------------------------
# Runner: compile once per process, execute via PJRT on 8 cores.
# ---------------------------------------------------------------------------
_RUNNER = None


def _make_runner(nc, n_cores=N_CORES):
    import jax
    from jax.sharding import Mesh, PartitionSpec, NamedSharding
    from jax.experimental.shard_map import shard_map
    from concourse import bass2jax

    bass2jax.install_neuronx_cc_hook()
    partition_name = nc.partition_id_tensor.name if nc.partition_id_tensor else None
    in_names, out_names, out_avals, out_shapes = [], [], [], []
    for alloc in nc.m.functions[0].allocations:
        if not isinstance(alloc, mybir.MemoryLocationSet):
            continue
        name = alloc.memorylocations[0].name
        if alloc.kind == "ExternalInput":
            if name != partition_name:
                in_names.append(name)
        elif alloc.kind == "ExternalOutput":
            out_names.append(name)
            shape = tuple(alloc.tensor_shape)
            dtype = mybir.dt.np(alloc.dtype)
            out_avals.append(jax.core.ShapedArray(shape, dtype))
            out_shapes.append((shape, dtype))
    n_params = len(in_names)
    all_in_names = list(in_names) + list(out_names)
    if partition_name is not None:
        all_in_names.append(partition_name)

    def _body(*args):
        operands = list(args)
        if partition_name is not None:
            operands.append(bass2jax.partition_id_tensor())
        outs = bass2jax._bass_exec_p.bind(
            *operands,
            out_avals=tuple(out_avals),
            in_names=tuple(all_in_names),
            out_names=tuple(out_names),
            lowering_input_output_aliases=(),
            sim_require_finite=True,
            sim_require_nnan=True,
            nc=nc,
        )
        return tuple(outs)

    devices = jax.devices()[:n_cores]
    mesh = Mesh(np.asarray(devices), ("core",))
    n_outs = len(out_names)
    in_specs = (PartitionSpec("core"),) * (n_params + n_outs)
    out_specs = (PartitionSpec("core"),) * n_outs
    sharded = jax.jit(
        shard_map(_body, mesh=mesh, in_specs=in_specs, out_specs=out_specs,
                  check_rep=False),
        keep_unused=True,
    )
    sh = NamedSharding(mesh, PartitionSpec("core"))

    def run(in_maps):
        per_core = [[np.ascontiguousarray(m[name]) for name in in_names]
                    for m in in_maps]
        concat_in = [
            np.concatenate([per_core[c][i] for c in range(n_cores)], axis=0)
            for i in range(n_params)
        ]
        concat_zeros = [
            np.zeros((n_cores * s[0], *s[1:]), d) for (s, d) in out_shapes
        ]
        dev_args = [jax.device_put(x, sh) for x in concat_in + concat_zeros]
        out_arrs = sharded(*dev_args)
        jax.block_until_ready(out_arrs)
        return [
            {name: np.asarray(out_arrs[i]).reshape(n_cores, *out_shapes[i][0])[c]
             for i, name in enumerate(out_names)}
            for c in range(n_cores)
        ], (sharded, dev_args, out_names, out_shapes)

    return run


def _rope_cache_T(position_ids):
    """cosT/sinT [128, S] per batch row; sinT pre-signed for rotate_half."""
    inv_freq = 1.0 / (ROPE_BASE ** (np.arange(0, HD, 2, dtype=np.float32) / HD))
    caches = []
    for b in range(position_ids.shape[0]):
        pos = np.asarray(position_ids[b], dtype=np.float32)        # [S]
        freqs = np.outer(pos, inv_freq).astype(np.float32)          # [S, 64]
        emb = np.concatenate([freqs, freqs], axis=-1)               # [S, 128]
        cos = np.cos(emb).astype(np.float32).T                      # [128, S]
        sin = np.sin(emb).astype(np.float32).T
        sin_signed = sin.copy()
        sin_signed[0:64, :] *= -1.0
        caches.append((np.ascontiguousarray(cos), np.ascontiguousarray(sin_signed)))
    return caches


def _shard_inputs(hidden_states, position_ids, wq, wk, wv, wo):
    x = np.asarray(hidden_states, dtype=np.float32)
    caches = _rope_cache_T(np.asarray(position_ids))
    in_maps = []
    for c in range(N_CORES):
        b = c // (N_CORES // B)
        hb = c % (N_CORES // B)
        rows = slice(hb * HB, (hb + 1) * HB)
        xT = np.ascontiguousarray(x[b].T).astype(MM_NP)
        in_maps.append({
            "xT": xT,
            "wqT": np.ascontiguousarray(np.asarray(wq)[rows, :].T).astype(MM_NP),
            "wkT": np.ascontiguousarray(np.asarray(wk)[rows, :].T).astype(MM_NP),
            "wvT": np.ascontiguousarray(np.asarray(wv)[rows, :].T).astype(MM_NP),
            "woT": np.ascontiguousarray(np.asarray(wo)[:, rows].T).astype(MM_NP),
            "cosT": caches[b][0],
            "sinT": caches[b][1],
            "ones": np.ones((128, 128), dtype=MM_NP),
        })
    return in_maps


def kernel(hidden_states, attention_mask, position_ids, wq, wk, wv, wo):
    global _RUNNER
    if _RUNNER is None:
        nc = _build_module()
        _RUNNER = _make_runner(nc)
    in_maps = _shard_inputs(hidden_states, position_ids, wq, wk, wv, wo)
    results, _ = _RUNNER(in_maps)
    g = N_CORES // B
    out = np.stack([
        np.sum([results[b * g + i]["out"] for i in range(g)], axis=0)
        for b in range(B)
    ]).astype(np.float32)
    return out
